# revision 10
# baseline (speedup 1.0000x reference)
"""Trainium2 Bass kernel for nn_NodeTreeFunc (gnn_message_passing).

Math per node i (see reference):
    ea_i  = edge_attr rows for node i, grouped by dest col  -> [D=16, 128]
    d0    = relu(cat[ea_i, x_i]) @ We + be                  -> [16, 128]
    4x tree level (same Ws1/Ws2 each level):
        h   = relu(cat[d_2j, d_2j+1, x_i]) @ Ws1 + bs1      -> [n2, 256]
        d   = relu(h) @ Ws2 + bs2                           -> [n2, 128]
    node mlp + residual:
        m   = relu(cat[x_i, d4]) @ Wm1 + bm1
        out = relu(m) @ Wm2 + bm2 + x_i

Mapping: nodes sharded across 8 cores (data parallel, no collectives);
feature-major on device ([channel -> partition, node -> free]). The K=256
contractions run as fp8e4 DoubleRow matmuls (HW-measured ~243-248 ns vs
~231 ns for a K=128 bf16 matmul at N=512 -> ~1.9x PE throughput; the
baseline all-bf16 kernel needed 155 K=128 MMs per 512-node tile, this
one runs 61 DR + 33 bf16):
  - encode: DR pair (relu(ea_r), relu(x)) @ (We0; We1) - one DR MM per
    edge row; the x-term contracts for free as the second K-half (the
    host appends a relu(x) fp8 row to each node's edge rows, and the DR
    rhs is a stride-(16-r) pair slice of that [17, T] block).
  - tree h: DR pair (d_even, d_odd) @ (Ws1a; Ws1b); the per-node x-term
    stays a bf16 K=128 MM into the same PSUM accumulation group.
  - tree d: DR pair (h_m0, h_m1) @ (Ws2a; Ws2b).
Intermediates d0..d3 and h are drained straight to fp8e4: an fp32 numpy
sim of this exact quantization (incl. e4m3 weight subnormals) gives l2
rel err 5.8e-4 vs the 2e-2 gate - the tree is contractive and the fp32
x-residual dominates the output. Level-4 output, node mlp, and residual
stay bf16/fp32. Measured on HW: rel err 5.85e-4.

Each stage ends with one fused bias+relu PSUM->SBUF drain, greedily
balanced between ScalarE ((80+fd)/1.2 ns) and VectorE ((140+fd)/0.96 ns)
(constants HW-measured; both engines cap at 1 col/cycle from PSUM, so
the ~32k drained columns per tile cost ~15us/tile across both engines,
just under the PE's ~17us). Two node tiles are software-pipelined
group-by-group (generator interleave) so the serial tree tail of one
tile hides behind the dense head of the other; PSUM runs as 4 rotating
2-bank slots.

Measured steady-state (17-iter slope, async k-sweep): ~223us/pass vs
the bf16 baseline's ~372us on the same protocol (1.67x). Variants tried
and rejected by measurement: custom fused add-relu DVE drains to remove
the 30 h x-matmuls (3-engine balance looked ~180us on paper but
scheduling stalls made it 244us), gpsimd y-copies (DVE<->GpSimd shared
SBUF port lock), h MM reorder DR,DR,x,x (regressed vs alternating).
"""

import numpy as np
import ml_dtypes

import concourse.bacc as bacc
import concourse.bass as bass
import concourse.mybir as mybir
from concourse.bass import ts
from concourse.bass_utils import run_bass_kernel_spmd
from concourse.tile import TileContext

N, D, CH = 40000, 16, 128
NCORES = 8
NC_NODES = N // NCORES      # 5000 nodes per core
T = 512                     # nodes per on-device tile
NT = (NC_NODES + T - 1) // T
NPAD = NT * T               # 5120 (padded with zero nodes)
DX = D + 1                  # ea rows + the relu(x) row

F32 = mybir.dt.float32
BF16 = mybir.dt.bfloat16
FP8 = mybir.dt.float8e4
BF16_NP = ml_dtypes.bfloat16
FP8_NP = ml_dtypes.float8_e4m3fn

DR = mybir.MatmulPerfMode.DoubleRow

# DR weight chunk indices inside wq [128, 2, 4*128]
WQ_E, WQ_H0, WQ_H1, WQ_D = 0, 1, 2, 3
# bf16 weight chunk indices inside wb [128, 5*128]
WB_S1X0, WB_S1X1, WB_M1A, WB_M1B, WB_M2 = 0, 1, 2, 3, 4
# bias columns inside bp [128, 8]
B_E, B_S1A, B_S1B, B_S2, B_M1, B_M2 = 0, 1, 2, 3, 4, 5

TRACE = False
LAST_RESULT = None


def _build_program(iters=1):
    nc = bacc.Bacc()
    eaq = nc.declare_dram_parameter("eaq", [128, NT * DX * T], FP8, isOutput=False)
    xT = nc.declare_dram_parameter("xT", [128, NPAD], F32, isOutput=False)
    wq = nc.declare_dram_parameter("wq", [128, 2 * 4 * 128], FP8, isOutput=False)
    wb = nc.declare_dram_parameter("wb", [128, 5 * 128], BF16, isOutput=False)
    bp = nc.declare_dram_parameter("bp", [128, 8], F32, isOutput=False)
    outT = nc.declare_dram_parameter("outT", [128, NPAD], F32, isOutput=True)

    relu = mybir.ActivationFunctionType.Relu
    ident = mybir.ActivationFunctionType.Identity
    add_op = mybir.AluOpType.add
    max_op = mybir.AluOpType.max

    eng_cost = {"act": 0.0, "dve": 0.0}

    with TileContext(nc) as tc:
        with (
            tc.tile_pool(name="consts", bufs=1) as consts,
            tc.tile_pool(name="eap", bufs=2) as ea_pool,
            tc.tile_pool(name="io", bufs=3) as io_pool,
            tc.tile_pool(name="mids", bufs=2) as mids,
            tc.tile_pool(name="psum", bufs=4, space="PSUM") as psum_pool,
        ):
            wq3 = consts.tile([128, 2, 4 * 128], FP8)
            nc.sync.dma_start(wq3[:, :, :], wq[:, :])
            wb_sb = consts.tile([128, 5 * 128], BF16)
            nc.sync.dma_start(wb_sb[:], wb[:, :])
            b_sb = consts.tile([128, 8], F32)
            nc.sync.dma_start(b_sb[:], bp[:, :])

            def bias(col):
                return b_sb[:, col : col + 1]

            def wdr(idx):
                return wq3[:, :, ts(idx, 128)]

            def wbf(idx):
                return wb_sb[:, ts(idx, 128)]

            def drain(out_ap, psum_ap, bias_col, fd):
                c_act = (80.0 + fd) / 1.2
                c_dve = (140.0 + fd) / 0.96
                if eng_cost["act"] + c_act <= eng_cost["dve"] + c_dve:
                    eng_cost["act"] += c_act
                    nc.scalar.activation(out_ap, psum_ap, relu, bias=bias(bias_col))
                else:
                    eng_cost["dve"] += c_dve
                    nc.vector.tensor_scalar(
                        out=out_ap,
                        in0=psum_ap,
                        scalar1=bias(bias_col),
                        scalar2=0.0,
                        op0=add_op,
                        op1=max_op,
                    )

            def tile_body(i):
                # ---- load node tile ----
                ea3 = ea_pool.tile([128, DX, T], FP8, tag="ea3")
                nc.sync.dma_start(ea3[:, :, :], eaq[:, ts(i, DX * T)])

                xt = io_pool.tile([128, T], F32, tag="xt")
                nc.sync.dma_start(xt[:], xT[:, ts(i, T)])
                xr = io_pool.tile([128, T], BF16, tag="xr")
                nc.vector.tensor_scalar_max(xr[:], xt[:], 0.0)
                xb = io_pool.tile([128, T], F32, tag="xb")
                nc.scalar.activation(xb[:], xt[:], ident, bias=bias(B_M2))
                eng_cost["dve"] += (58.0 + T / 2.0) / 0.96
                eng_cost["act"] += (224.0 + T) / 1.2
                yield

                # ---- encode: one DR MM per edge row (x fused) ----
                d0 = mids.tile([128, D, T], FP8, tag="d0")
                for g in range(8):
                    ps = psum_pool.tile([128, 2 * T], F32, tag="ps")
                    for j in range(2):
                        r = 2 * g + j
                        nc.tensor.matmul(
                            ps[:, ts(j, T)], wdr(WQ_E),
                            ea3[:, r : DX : DX - 1 - r, :],
                            start=True, stop=True, perf_mode=DR,
                        )
                    drain(d0[:, 2 * g : 2 * g + 2, :], ps[:, : 2 * T], B_E, 2 * T)
                    yield

                # ---- tree levels ----
                prev = d0
                rows = D
                lvl = 0
                while rows > 1:
                    r2 = rows // 2
                    lvl += 1
                    h3 = mids.tile([128, 2, r2 * T], FP8, tag=f"h{lvl}")
                    for m in range(2):
                        for jg in range(0, r2, 2):
                            gw = min(2, r2 - jg)
                            ps = psum_pool.tile([128, 2 * T], F32, tag="ps")
                            for jj in range(gw):
                                j = jg + jj
                                nc.tensor.matmul(
                                    ps[:, ts(jj, T)], wdr(WQ_H0 + m),
                                    prev[:, 2 * j : 2 * j + 2, :],
                                    start=True, stop=False, perf_mode=DR,
                                )
                                nc.tensor.matmul(
                                    ps[:, ts(jj, T)], wbf(WB_S1X0 + m), xr[:],
                                    start=False, stop=True,
                                )
                            drain(h3[:, m, jg * T : (jg + gw) * T], ps[:, : gw * T],
                                  B_S1A if m == 0 else B_S1B, gw * T)
                            yield
                    if r2 > 1:
                        dn = mids.tile([128, r2, T], FP8, tag=f"d{lvl}")
                    else:
                        dn = mids.tile([128, 1, T], BF16, tag=f"d{lvl}")
                    for jg in range(0, r2, 2):
                        gw = min(2, r2 - jg)
                        ps = psum_pool.tile([128, 2 * T], F32, tag="ps")
                        for jj in range(gw):
                            j = jg + jj
                            nc.tensor.matmul(
                                ps[:, ts(jj, T)], wdr(WQ_D),
                                h3[:, :, j * T : (j + 1) * T],
                                start=True, stop=True, perf_mode=DR,
                            )
                        drain(dn[:, jg : jg + gw, :], ps[:, : gw * T], B_S2, gw * T)
                        yield
                    prev = dn
                    rows = r2

                # ---- node mlp + residual ----
                ps = psum_pool.tile([128, 2 * T], F32, tag="ps")
                nc.tensor.matmul(ps[:, :T], wbf(WB_M1A), xr[:], start=True, stop=False)
                nc.tensor.matmul(ps[:, :T], wbf(WB_M1B), prev[:, 0, :],
                                 start=False, stop=True)
                mh = io_pool.tile([128, T], BF16, tag="mh")
                drain(mh[:], ps[:, :T], B_M1, T)
                yield

                ps2 = psum_pool.tile([128, 2 * T], F32, tag="ps")
                nc.tensor.matmul(ps2[:, :T], wbf(WB_M2), mh[:], start=True, stop=True)
                outf = io_pool.tile([128, T], F32, tag="outf")
                nc.vector.tensor_add(outf[:], ps2[:, :T], xb[:])
                eng_cost["dve"] += (151.0 + T) / 0.96
                nc.sync.dma_start(outT[:, ts(i, T)], outf[:])
                yield

            # drive two node tiles interleaved group-by-group
            order = [i for _ in range(iters) for i in range(NT)]
            from collections import deque
            pending = deque(order)
            active = deque()
            while pending or active:
                while len(active) < 2 and pending:
                    active.append(tile_body(pending.popleft()))
                gen = active.popleft()
                try:
                    next(gen)
                    active.append(gen)
                except StopIteration:
                    pass

    nc.finalize()
    return nc


_PROG = None


def _get_prog():
    global _PROG
    if _PROG is None:
        _PROG = _build_program()
    return _PROG


def _q8(a):
    return np.clip(a, -240.0, 240.0).astype(FP8_NP)


def _prepare_in_maps(x, edge_index, edge_attr, We, be, Ws1, bs1, Ws2, bs2,
                     Wm1, bm1, Wm2, bm2):
    x = np.asarray(x, dtype=np.float32)
    edge_attr = np.asarray(edge_attr, dtype=np.float32)
    assert x.shape == (N, CH) and edge_attr.shape == (N * D, CH)

    # group edges by destination column; identity for the canonical layout
    col = np.asarray(edge_index)[1]
    if not np.array_equal(col, np.repeat(np.arange(N, dtype=col.dtype), D)):
        edge_attr = edge_attr[np.argsort(col, kind="stable")]

    # relu + fp8 quantization for the DR moving operands
    ea_q = _q8(np.maximum(edge_attr, 0.0))
    x_q = _q8(np.maximum(x, 0.0))

    We = np.asarray(We, np.float32)
    Ws1 = np.asarray(Ws1, np.float32)
    Ws2 = np.asarray(Ws2, np.float32)
    Wm1 = np.asarray(Wm1, np.float32)
    Wm2 = np.asarray(Wm2, np.float32)

    def _sl(c):
        return slice(c * 128, (c + 1) * 128)

    # DR weights: [128, 2, 4*128] - K-halves along dim 1
    wq_pack = np.zeros((128, 2, 4 * 128), np.float32)
    wq_pack[:, 0, _sl(WQ_E)] = We[0:128]
    wq_pack[:, 1, _sl(WQ_E)] = We[128:256]
    for m in range(2):
        wq_pack[:, 0, _sl(WQ_H0 + m)] = Ws1[0:128, m * 128:(m + 1) * 128]
        wq_pack[:, 1, _sl(WQ_H0 + m)] = Ws1[128:256, m * 128:(m + 1) * 128]
    wq_pack[:, 0, _sl(WQ_D)] = Ws2[0:128]
    wq_pack[:, 1, _sl(WQ_D)] = Ws2[128:256]
    wq_pack = _q8(wq_pack).reshape(128, -1)

    wb_chunks = [Ws1[256:384, 0:128], Ws1[256:384, 128:256],
                 Wm1[0:128], Wm1[128:256], Wm2]
    wb_pack = np.ascontiguousarray(
        np.concatenate(wb_chunks, axis=1)).astype(BF16_NP)

    bpack = np.zeros((128, 8), np.float32)
    bpack[:, B_E] = np.asarray(be, np.float32)
    bpack[:, B_S1A] = np.asarray(bs1, np.float32)[0:128]
    bpack[:, B_S1B] = np.asarray(bs1, np.float32)[128:256]
    bpack[:, B_S2] = np.asarray(bs2, np.float32)
    bpack[:, B_M1] = np.asarray(bm1, np.float32)
    bpack[:, B_M2] = np.asarray(bm2, np.float32)

    in_maps = []
    for c in range(NCORES):
        ea_c = ea_q[c * NC_NODES * D : (c + 1) * NC_NODES * D].reshape(
            NC_NODES, D, CH)
        x_c = x[c * NC_NODES : (c + 1) * NC_NODES]
        xq_c = x_q[c * NC_NODES : (c + 1) * NC_NODES]
        if NPAD != NC_NODES:
            pad = NPAD - NC_NODES
            ea_c = np.concatenate(
                [ea_c, np.zeros((pad, D, CH), FP8_NP)], axis=0)
            x_c = np.concatenate([x_c, np.zeros((pad, CH), np.float32)], 0)
            xq_c = np.concatenate([xq_c, np.zeros((pad, CH), FP8_NP)], 0)
        # [node, DX, ch]: ea rows then the relu(x) row
        comb = np.concatenate([ea_c, xq_c[:, None, :]], axis=1)
        # -> [ch, tile, r, t] feature-major, node-tiles outermost
        ea_t = np.ascontiguousarray(
            comb.reshape(NT, T, DX, CH).transpose(3, 0, 2, 1)
        ).reshape(128, NT * DX * T)
        xT_c = np.ascontiguousarray(x_c.T)
        in_maps.append({"eaq": ea_t, "xT": xT_c, "wq": wq_pack,
                        "wb": wb_pack, "bp": bpack})

    return in_maps


def kernel(**inputs):
    global LAST_RESULT
    in_maps = _prepare_in_maps(**inputs)
    res = run_bass_kernel_spmd(_get_prog(), in_maps, list(range(NCORES)), trace=TRACE)
    LAST_RESULT = res
    outs = [res.results[c]["outT"].T[:NC_NODES] for c in range(NCORES)]
    return np.ascontiguousarray(np.concatenate(outs, axis=0), dtype=np.float32)
